# revision 1
# baseline (speedup 1.0000x reference)
"""CrossViewFusion Trainium2 kernel.

Math (per batch row b):
  seq = [x_cc; x_mlo]                  # 2 views, D=512 each
  qkv = seq @ in_proj_w.T + b          # per view: q,k,v (512 each, 8 heads x 64)
  scores[h,qv,kv] = q_qv[h] . k_kv[h] / 8
  key mlo masked out when view_mask[:,1]==0 -> softmax over 2 keys
  ao = attn @ v ; proj = ao @ out_w.T + out_b
  h = seq + proj ; t = LN(h) (gamma/beta = ln_g/ln_b)
  g = sigmoid([t_cc*g+b ; t_mlo*g+b] @ gate_w.T + gate_b)
  fused = g*cc + (1-g)*mlo ; out = has_mlo ? fused : cc   (has_cc==1)

Implementation (per core, B/8 = 8192 rows, 16 super-groups of 4x128 rows):
  - row-major activations on 128 partitions; bf16 in SBUF, fp32 PSUM/stats
  - matmuls activation-stationary: lhsT = transposed-activation chunk
    [128 D, 128 rows], rhs = pre-transposed weights (moving), out row-major
  - layout transposes via multi-block DMA xbar (one instr per [128,2048])
  - batched DMA: one load/store per super-group tensor
  - softmax over 2 keys folded to sigmoid: w = sigma((s_cc - s_mlo)/8)
  - masking folded into per-row scalars (w_eff, blend coefficients)
  - ln_g folded into gate weights host-side; trivial affine (ln_g=1,ln_b=0)
    and zero biases skip their ops; general fallback IR otherwise
"""

import sys

for _p in ("/opt/trn_rl_repo",):
    if _p not in sys.path:
        sys.path.append(_p)

import numpy as np
import ml_dtypes

B, D, H, HD = 65536, 512, 8, 64
NCORES = 8
BS = B // NCORES          # rows per core
P = 128                   # SBUF partitions
SG = 2                    # row-groups per super-group
EPS = 1e-5

BF16 = ml_dtypes.bfloat16

_cache = {}


def _build(flags, bs=BS, repeats=1):
    """Build + compile the per-core Bass kernel. flags =
    (zero_ipb, zero_ob, zero_gb2, unit_lng, zero_lnb, mask_binary).
    repeats>1 wraps the body in a hardware loop (benchmarking only)."""
    import concourse.mybir as mybir
    from concourse import bacc, tile
    from concourse.bass import ts
    from contextlib import ExitStack

    zero_ipb, zero_ob, zero_gb2, unit_lng, zero_lnb, mask_binary = flags
    blend_then_affine = zero_lnb or mask_binary

    nsg = bs // (P * SG)
    W = SG * D                 # big-tile free width (2048)
    f32 = mybir.dt.float32
    bf16 = mybir.dt.bfloat16
    AF = mybir.ActivationFunctionType
    OP = mybir.AluOpType
    AX = mybir.AxisListType

    nc = bacc.Bacc("TRN2", target_bir_lowering=False, debug=False,
                   enable_asserts=False)

    # ---- DRAM I/O ----
    x_cc_d = nc.dram_tensor("x_cc", [bs, D], f32, kind="ExternalInput").ap()
    x_mlo_d = nc.dram_tensor("x_mlo", [bs, D], f32, kind="ExternalInput").ap()
    vm_d = nc.dram_tensor("vm", [bs, 2], f32, kind="ExternalInput").ap()
    wqkvT_d = nc.dram_tensor("wqkvT", [D, 3 * D], bf16, kind="ExternalInput").ap()
    owT_d = nc.dram_tensor("owT", [D, D], bf16, kind="ExternalInput").ap()
    gwT_d = nc.dram_tensor("gwT", [2 * D, D], bf16, kind="ExternalInput").ap()
    opt_in = {}
    if not zero_ipb:
        opt_in["ipb"] = nc.dram_tensor("ipb", [3 * D], f32, kind="ExternalInput").ap()
    if not zero_ob:
        opt_in["ob"] = nc.dram_tensor("ob", [D], f32, kind="ExternalInput").ap()
    if not zero_gb2:
        opt_in["gb2"] = nc.dram_tensor("gb2", [D], f32, kind="ExternalInput").ap()
    if not unit_lng:
        opt_in["lng"] = nc.dram_tensor("lng", [D], f32, kind="ExternalInput").ap()
    if not zero_lnb:
        opt_in["lnb"] = nc.dram_tensor("lnb", [D], f32, kind="ExternalInput").ap()
    out_d = nc.dram_tensor("out", [bs, D], f32, kind="ExternalOutput").ap()

    with tile.TileContext(nc) as tc, ExitStack() as ctx:
        wpool = ctx.enter_context(tc.tile_pool(name="wpool", bufs=1))
        sb = ctx.enter_context(tc.tile_pool(name="sb", bufs=2))
        ps_qk = ctx.enter_context(tc.tile_pool(name="ps_qk", bufs=4, space="PSUM"))
        ps_v = ctx.enter_context(tc.tile_pool(name="ps_v", bufs=2, space="PSUM"))
        ps_pg = ctx.enter_context(tc.tile_pool(name="ps_pg", bufs=2, space="PSUM"))

        # ---- resident weights ----
        wqkvT_sb = wpool.tile([P, 4, 3 * D], bf16)
        nc.sync.dma_start(wqkvT_sb[:], wqkvT_d.rearrange("(c p) f -> p c f", p=P))
        owT_sb = wpool.tile([P, 4, D], bf16)
        nc.sync.dma_start(owT_sb[:], owT_d.rearrange("(c p) f -> p c f", p=P))
        gwT_sb = wpool.tile([P, 8, D], bf16)
        nc.sync.dma_start(gwT_sb[:], gwT_d.rearrange("(c p) f -> p c f", p=P))

        def bcast_tile(name, dram_ap, n, dtype):
            t32 = wpool.tile([P, n], f32, name=name + "_f32")
            nc.sync.dma_start(t32[:], dram_ap[None, :].to_broadcast((P, n)))
            if dtype == f32:
                return t32
            tb = wpool.tile([P, n], dtype, name=name + "_bf")
            nc.scalar.copy(tb[:], t32[:])
            return tb

        eps_p1 = wpool.tile([P, 1], f32)
        nc.vector.memset(eps_p1[:], EPS)

        # PE warmup: scratch matmuls with no DMA dependency keep the HAM
        # clock-gate warm while the weight loads land; results discarded.
        wu_s = wpool.tile([P, D], bf16)
        nc.vector.memset(wu_s[:], 0)
        if repeats == 1:
            wu_ps = ps_pg.tile([P, D], f32, name="wu_ps", tag="pg")
            for _ in range(20):
                nc.tensor.matmul(wu_ps[:], wu_s[:, 0:P], wu_s[:],
                                 start=True, stop=True)

        ipb_bc = None if zero_ipb else bcast_tile("ipb_bc", opt_in["ipb"], 3 * D, f32)
        ob_bc = None if zero_ob else bcast_tile("ob_bc", opt_in["ob"], D, f32)
        gb2_bc = None if zero_gb2 else bcast_tile("gb2_bc", opt_in["gb2"], D, f32)
        lng_bc = None if unit_lng else bcast_tile("lng_bc", opt_in["lng"], D, f32)
        lnb_bc = None if zero_lnb else bcast_tile("lnb_bc", opt_in["lnb"], D, f32)

        rep_cm = tc.For_i(0, repeats, 1) if repeats > 1 else None
        if rep_cm is not None:
            rep_cm.__enter__()

        for s in range(nsg):
            rows = ts(s, P * SG)
            # ---- batched loads ----
            xccF = sb.tile([P, SG, D], f32, bufs=3)
            nc.sync.dma_start(xccF[:], x_cc_d[rows, :].rearrange(
                "(n p) d -> p n d", p=P))
            xmloF = sb.tile([P, SG, D], f32, bufs=3)
            nc.sync.dma_start(xmloF[:], x_mlo_d[rows, :].rearrange(
                "(n p) d -> p n d", p=P))
            vmB = sb.tile([P, SG, 2], f32)
            nc.sync.dma_start(vmB[:], vm_d[rows, :].rearrange(
                "(n p) c -> p n c", p=P))

            # ---- per-row mask scalars, [128, SG] ----
            a4 = vmB[:, :, 0]
            m4 = vmB[:, :, 1]
            bm4 = sb.tile([P, SG], f32)
            nc.vector.tensor_scalar(bm4[:], m4, 0.0, None, op0=OP.not_equal)
            onemb4 = sb.tile([P, SG], f32)
            nc.vector.tensor_scalar(onemb4[:], bm4[:], -1.0, 1.0, op0=OP.mult,
                                    op1=OP.add)
            am4 = sb.tile([P, SG], f32)
            nc.vector.tensor_mul(am4[:], a4, m4)
            c4 = sb.tile([P, SG], f32)
            nc.vector.tensor_scalar(c4[:], am4[:], 0.5, None, op0=OP.is_gt)
            u4 = sb.tile([P, SG], f32)
            nc.vector.tensor_scalar(u4[:], c4[:], -1.0, 1.0, op0=OP.mult,
                                    op1=OP.add)
            scc24 = sb.tile([P, SG], f32)
            nc.vector.tensor_mul(scc24[:], u4[:], a4)
            negc4 = sb.tile([P, SG], f32)
            nc.vector.tensor_scalar(negc4[:], c4[:], -1.0, None, op0=OP.mult)
            mu4 = sb.tile([P, SG], f32)
            nc.vector.tensor_mul(mu4[:], m4, u4[:])
            smlo24 = sb.tile([P, SG], f32)
            nc.vector.tensor_add(smlo24[:], mu4[:], c4[:])

            # ---- bf16 cast + one xbar transpose per view ----
            xTs = []
            for xvF, nm in ((xccF, "cc"), (xmloF, "mlo")):
                xbf = sb.tile([P, SG, D], bf16, name=f"xbf_{nm}")
                nc.any.tensor_copy(xbf[:], xvF[:])
                xT = sb.tile([P, SG * 4, P], bf16, name=f"xT_{nm}", bufs=3)
                nc.sync.dma_start_transpose(
                    xT[:], xbf[:].rearrange("p n d -> p (n d)"))
                xTs.append(xT)

            # big bf16 intermediates
            dkB = sb.tile([P, SG, D], bf16)
            dvB = sb.tile([P, SG, D], bf16)
            pccB = sb.tile([P, SG, D], bf16)
            pmloB = sb.tile([P, SG, D], bf16)
            aoB = [sb.tile([P, SG, D], bf16, name=f"ao_{nm}")
                   for nm in ("cc", "mlo")]
            wdvB = [sb.tile([P, SG, D], bf16, name=f"wdv_{nm}")
                    for nm in ("cc", "mlo")]
            sAll = sb.tile([P, 2, SG, H], f32)
            hB = [sb.tile([P, SG, D], bf16, name=f"h_{nm}")
                  for nm in ("cc", "mlo")]
            hsum8 = sb.tile([P, 2 * SG], f32)
            sq8 = sb.tile([P, 2 * SG], f32)
            tB = [sb.tile([P, SG, D], bf16, name=f"t_{nm}")
                  for nm in ("cc", "mlo")]
            gsigB = sb.tile([P, SG, D], bf16)
            bccB = sb.tile([P, SG, D], bf16)
            bmlB = sb.tile([P, SG, D], bf16)
            ofinB = sb.tile([P, SG, D], f32)

            # ---- per-group in_proj matmuls; ACT drains PSUM to SBUF ----
            qkS = [sb.tile([P, SG, 2 * D], bf16, name=f"qkS_{nm}", bufs=3)
                   for nm in ("cc", "mlo")]
            vS = [sb.tile([P, SG, D], bf16, name=f"vS_{nm}", bufs=3)
                  for nm in ("cc", "mlo")]
            for n in range(SG):
                for vi in range(2):
                    for j in range(2):
                        pqk = ps_qk.tile([P, D], f32, name="pqk", tag="pqk")
                        for c in range(4):
                            nc.tensor.matmul(
                                pqk[:], xTs[vi][:, 4 * n + c, :],
                                wqkvT_sb[:, c, ts(j, D)],
                                start=(c == 0), stop=(c == 3))
                        if not zero_ipb:
                            nc.vector.tensor_add(pqk[:], pqk[:],
                                                 ipb_bc[:, ts(j, D)])
                        nc.any.tensor_copy(qkS[vi][:, n, ts(j, D)], pqk[:])
                    pv = ps_v.tile([P, D], f32, name="pv", tag="pv")
                    for c in range(4):
                        nc.tensor.matmul(
                            pv[:], xTs[vi][:, 4 * n + c, :],
                            wqkvT_sb[:, c, 2 * D:3 * D],
                            start=(c == 0), stop=(c == 3))
                    if not zero_ipb:
                        nc.vector.tensor_add(pv[:], pv[:], ipb_bc[:, 2 * D:])
                    nc.any.tensor_copy(vS[vi][:, n, :], pv[:])
            # big SBUF attention TTs (2x mode)
            nc.vector.tensor_sub(dkB[:], qkS[0][:, :, D:2 * D],
                                 qkS[1][:, :, D:2 * D])
            nc.vector.tensor_mul(pccB[:], qkS[0][:, :, 0:D], dkB[:])
            nc.vector.tensor_mul(pmloB[:], qkS[1][:, :, 0:D], dkB[:])
            nc.vector.tensor_sub(dvB[:], vS[0][:], vS[1][:])

            # ---- scores -> sigmoid -> w_eff (big) ----
            nc.vector.reduce_sum(
                sAll[:, 0, :, :],
                pccB[:].rearrange("p n (h e) -> p (n h) e", e=HD), axis=AX.X)
            nc.vector.reduce_sum(
                sAll[:, 1, :, :],
                pmloB[:].rearrange("p n (h e) -> p (n h) e", e=HD), axis=AX.X)
            wsig = sb.tile([P, 2, SG, H], bf16)
            nc.scalar.activation(
                wsig[:].rearrange("p a n h -> p (a n h)"),
                sAll[:].rearrange("p a n h -> p (a n h)"),
                AF.Sigmoid, scale=1.0 / np.sqrt(HD))
            weff = sb.tile([P, 2, SG, H], bf16)
            nc.vector.tensor_mul(
                weff[:], wsig[:],
                bm4[:].unsqueeze(1).unsqueeze(3).broadcast_to((P, 2, SG, H)))
            nc.vector.tensor_add(
                weff[:], weff[:],
                onemb4[:].unsqueeze(1).unsqueeze(3).broadcast_to((P, 2, SG, H)))

            # ---- ao = v_mlo + w*(v_cc - v_mlo) ----
            for vi in range(2):
                nc.vector.tensor_mul(
                    wdvB[vi][:].rearrange("p n (h e) -> p n h e", e=HD),
                    dvB[:].rearrange("p n (h e) -> p n h e", e=HD),
                    weff[:, vi, :, :].unsqueeze(3).broadcast_to((P, SG, H, HD)))
                nc.vector.tensor_add(aoB[vi][:], wdvB[vi][:], vS[1][:])

            # ---- out_proj + residual + LN ----
            aoTs = []
            for vi, nm in ((0, "cc"), (1, "mlo")):
                aoT = sb.tile([P, SG * 4, P], bf16, name=f"aoT_{nm}")
                nc.sync.dma_start_transpose(
                    aoT[:], aoB[vi][:].rearrange("p n d -> p (n d)"))
                aoTs.append(aoT)
            for n in range(SG):
                for vi in range(2):
                    pp = ps_pg.tile([P, D], f32, name="pp", tag="pg")
                    for c in range(4):
                        nc.tensor.matmul(pp[:], aoTs[vi][:, 4 * n + c, :],
                                         owT_sb[:, c, :],
                                         start=(c == 0), stop=(c == 3))
                    if not zero_ob:
                        nc.vector.tensor_add(pp[:], pp[:], ob_bc[:])
                    xF = (xccF, xmloF)[vi]
                    nc.vector.tensor_add(hB[vi][:, n, :], xF[:, n, :], pp[:])
                    h2s = sb.tile([P, D], bf16, name="h2s", tag="h2s", bufs=2)
                    nc.scalar.activation(h2s[:], hB[vi][:, n, :],
                                         AF.Square,
                                         accum_out=sq8[:, vi * SG + n:
                                                       vi * SG + n + 1])
            nc.vector.reduce_sum(hsum8[:, 0:SG], hB[0][:], axis=AX.X)
            nc.vector.reduce_sum(hsum8[:, SG:2 * SG], hB[1][:], axis=AX.X)

            mneg8 = sb.tile([P, 2 * SG], f32)
            nc.vector.tensor_scalar(mneg8[:], hsum8[:], -1.0 / D, None,
                                    op0=OP.mult)
            ex28 = sb.tile([P, 2 * SG], f32)
            nc.vector.tensor_scalar(ex28[:], sq8[:], 1.0 / D, None, op0=OP.mult)
            var8 = sb.tile([P, 2 * SG], f32)
            nc.vector.tensor_mul(var8[:], mneg8[:], mneg8[:])
            nc.vector.tensor_sub(var8[:], ex28[:], var8[:])
            std8 = sb.tile([P, 2 * SG], f32)
            nc.scalar.activation(std8[:], var8[:], AF.Sqrt, bias=eps_p1[:])
            rs8 = sb.tile([P, 2 * SG], f32)
            nc.vector.reciprocal(rs8[:], std8[:])
            nmrs8 = sb.tile([P, 2 * SG], f32)
            nc.vector.tensor_mul(nmrs8[:], mneg8[:], rs8[:])

            for vi in range(2):
                for n in range(SG):
                    i8 = vi * SG + n
                    nc.vector.tensor_scalar(tB[vi][:, n, :], hB[vi][:, n, :],
                                            rs8[:, i8:i8 + 1],
                                            nmrs8[:, i8:i8 + 1],
                                            op0=OP.mult, op1=OP.add)

            # ---- gate ----
            tTs = []
            for vi, nm in ((0, "cc"), (1, "mlo")):
                tT = sb.tile([P, SG * 4, P], bf16, name=f"tT_{nm}")
                nc.sync.dma_start_transpose(
                    tT[:], tB[vi][:].rearrange("p n d -> p (n d)"))
                tTs.append(tT)
            for n in range(SG):
                pg = ps_pg.tile([P, D], f32, name="pgate", tag="pg")
                for c in range(4):
                    nc.tensor.matmul(pg[:], tTs[0][:, 4 * n + c, :],
                                     gwT_sb[:, c, :],
                                     start=(c == 0), stop=False)
                for c in range(4):
                    nc.tensor.matmul(pg[:], tTs[1][:, 4 * n + c, :],
                                     gwT_sb[:, 4 + c, :],
                                     start=False, stop=(c == 3))
                if not zero_gb2:
                    nc.vector.tensor_add(pg[:], pg[:], gb2_bc[:])
                nc.scalar.activation(gsigB[:, n, :], pg[:], AF.Sigmoid)
                nc.vector.tensor_scalar(bccB[:, n, :], gsigB[:, n, :],
                                        c4[:, n:n + 1], scc24[:, n:n + 1],
                                        op0=OP.mult, op1=OP.add)
                nc.vector.tensor_scalar(bmlB[:, n, :], gsigB[:, n, :],
                                        negc4[:, n:n + 1], smlo24[:, n:n + 1],
                                        op0=OP.mult, op1=OP.add)

            # ---- final blend (gpsimd) ----
            if blend_then_affine:
                o1 = sb.tile([P, SG, D], bf16)
                nc.gpsimd.tensor_mul(o1[:], bccB[:], tB[0][:])
                o2 = sb.tile([P, SG, D], bf16)
                nc.gpsimd.tensor_mul(o2[:], bmlB[:], tB[1][:])
                need_post = (not unit_lng) or (not zero_lnb)
                if not need_post:
                    nc.gpsimd.tensor_add(ofinB[:], o1[:], o2[:])
                else:
                    osum = sb.tile([P, SG, D], f32, name="osum")
                    nc.gpsimd.tensor_add(osum[:], o1[:], o2[:])
                    cur = osum
                    if not unit_lng:
                        for n in range(SG):
                            nc.vector.tensor_mul(cur[:, n, :], cur[:, n, :],
                                                 lng_bc[:])
                    if not zero_lnb:
                        for n in range(SG):
                            nc.vector.tensor_add(ofinB[:, n, :], cur[:, n, :],
                                                 lnb_bc[:])
                    else:
                        nc.vector.tensor_copy(ofinB[:], cur[:])
            else:
                fins = []
                for vi in range(2):
                    fv = sb.tile([P, SG, D], f32, name=f"fin{vi}")
                    for n in range(SG):
                        cur_in = tB[vi][:, n, :]
                        if not unit_lng:
                            nc.vector.tensor_mul(fv[:, n, :], cur_in, lng_bc[:])
                            cur_in = fv[:, n, :]
                        if not zero_lnb:
                            nc.vector.tensor_add(fv[:, n, :], cur_in, lnb_bc[:])
                        elif unit_lng:
                            nc.vector.tensor_copy(fv[:, n, :], cur_in)
                    fins.append(fv)
                o1 = sb.tile([P, SG, D], f32)
                nc.gpsimd.tensor_mul(o1[:], bccB[:], fins[0][:])
                o2 = sb.tile([P, SG, D], f32)
                nc.gpsimd.tensor_mul(o2[:], bmlB[:], fins[1][:])
                nc.gpsimd.tensor_add(ofinB[:], o1[:], o2[:])

            nc.sync.dma_start(
                out_d[rows, :].rearrange("(n p) d -> p n d", p=P), ofinB[:])

        if rep_cm is not None:
            rep_cm.__exit__(None, None, None)

    nc.compile()
    return nc


def _get_nc(flags, bs=BS):
    key = (flags, bs)
    if key not in _cache:
        _cache[key] = _build(flags, bs)
    return _cache[key]


def kernel(x_cc, x_mlo, view_mask, in_proj_w, in_proj_b, out_w, out_b,
           ln_g, ln_b, gate_w, gate_b):
    from concourse import bass_utils

    x_cc = np.asarray(x_cc, np.float32)
    x_mlo = np.asarray(x_mlo, np.float32)
    view_mask = np.asarray(view_mask, np.float32)
    in_proj_w = np.asarray(in_proj_w, np.float32)
    in_proj_b = np.asarray(in_proj_b, np.float32)
    out_w = np.asarray(out_w, np.float32)
    out_b = np.asarray(out_b, np.float32)
    ln_g = np.asarray(ln_g, np.float32)
    ln_b = np.asarray(ln_b, np.float32)
    gate_w = np.asarray(gate_w, np.float32)
    gate_b = np.asarray(gate_b, np.float32)

    # host-side weight prep (O(D^2), no per-row work)
    lng2 = np.concatenate([ln_g, ln_g])
    lnb2 = np.concatenate([ln_b, ln_b])
    gate_w_f = gate_w * lng2[None, :]
    gate_b_f = gate_b + gate_w @ lnb2
    wqkvT = np.ascontiguousarray(in_proj_w.T).astype(BF16)
    owT = np.ascontiguousarray(out_w.T).astype(BF16)
    gwT = np.ascontiguousarray(gate_w_f.T).astype(BF16)

    flags = (
        not in_proj_b.any(),
        not out_b.any(),
        not gate_b_f.any(),
        bool((ln_g == 1.0).all()),
        not ln_b.any(),
        bool(np.isin(view_mask, (0.0, 1.0)).all()),
    )
    nc = _get_nc(flags)

    in_maps = []
    for c in range(NCORES):
        sl = slice(c * BS, (c + 1) * BS)
        m = {
            "x_cc": x_cc[sl], "x_mlo": x_mlo[sl], "vm": view_mask[sl],
            "wqkvT": wqkvT, "owT": owT, "gwT": gwT,
        }
        zero_ipb, zero_ob, zero_gb2, unit_lng, zero_lnb, _ = flags
        if not zero_ipb:
            m["ipb"] = in_proj_b
        if not zero_ob:
            m["ob"] = out_b
        if not zero_gb2:
            m["gb2"] = gate_b_f
        if not unit_lng:
            m["lng"] = ln_g
        if not zero_lnb:
            m["lnb"] = ln_b
        in_maps.append(m)

    global _last_run
    _last_run = (nc, in_maps)
    res = bass_utils.run_bass_kernel_spmd(nc, in_maps, core_ids=list(range(NCORES)))
    return np.concatenate([r["out"] for r in res.results], axis=0)

